# revision 7
# baseline (speedup 1.0000x reference)
"""GCN conv block (gather -> normalized scatter-add -> matmul -> bias ->
LeakyReLU -> BatchNorm) on 8 Trainium2 NeuronCores.

v2 architecture (per core; SPMD single program, nodes sharded by range):
  - Prologue: dinv = 1/sqrt(1+in_degree) for all nodes from rowptr (device);
    per-core own-range dinv likewise.
  - Main loop over S source slabs. Slab fill: stream x node-tiles, scale by
    dinv (gpsimd), DMA-XBAR-transpose (2x 64-partition halves) into a resident
    f-major slab [128 f, SLAB nodes] in SBUF. Edge gathers then run as big
    gpsimd ap_gather ops (2048 edges each) producing f-major fragments; each
    128-edge chunk is PE-transposed to edge-major and multiplied with an
    on-the-fly one-hot R [e,d] = (dst_local==d) to segment-sum into per-tile
    PSUM, accumulated across slabs in an SBUF A_big [128 f, 12544 d].
  - Phase 3: per destination tile, add the self-loop term (dinv^2 x built via
    the same scale+transpose path from per-core x_own), scale columns by
    dinv[dst] (broadcast tile), apply W, bias, LeakyReLU (relu(z)-0.01relu(-z)
    with fused BN sum/sumsq accumulators).
  - Phase 4: AllReduce BN stats across the 8 cores, apply the affine, store
    out^T per tile. Host reassembles.
Host only shards/pads index structures (CSR bucketing) and reassembles.
"""
import sys

for _p in ("/opt/trn_rl_repo",):
    if _p not in sys.path:
        sys.path.insert(0, _p)

import numpy as np

from concourse import bass, bacc, mybir
import concourse.tile as tile
from concourse.bass_utils import run_bass_kernel_spmd
from concourse.masks import make_identity

P = 128
D = 128
N_CORES = 8
LEAKY = 0.01
BN_EPS = 1e-5
CHUNK = 128          # edges per matmul chunk
GATHER = 2048        # edges per ap_gather instruction
F32 = mybir.dt.float32
I16 = mybir.dt.int16

LAST_RESULTS = None


def _plan(counts_max, S, tpc):
    """counts_max: [S, tpc] max-over-cores edge counts per (slab, tile) cell.
    Returns per-slab chunk schedules: list (per s) of list of (tile, first, last),
    and group counts per slab."""
    k_cell = -(-counts_max // CHUNK)  # ceil; zeros stay zero
    sched = []
    groups = []
    for s in range(S):
        chunks = []
        for t in range(tpc):
            k = int(k_cell[s, t])
            for j in range(k):
                chunks.append((t, j == 0, j == k - 1))
        sched.append(chunks)
        groups.append(-(-len(chunks) // (GATHER // CHUNK)))
    return k_cell, sched, groups


def _build_program(n_nodes, tpc, slab_n, sched, groups):
    n_tab = ((n_nodes + P - 1) // P) * P
    ncol = n_tab // P
    n_own = tpc * P
    S = len(sched)
    g_tot = sum(groups)
    gpc = GATHER // CHUNK  # chunks per gather group

    nc = bacc.Bacc("TRN2", target_bir_lowering=False, debug=False)

    x_d = nc.dram_tensor("x", [n_tab, D], F32, kind="ExternalInput")
    xown_d = nc.dram_tensor("x_own", [n_own, D], F32, kind="ExternalInput")
    rowptr_d = nc.dram_tensor("rowptr", [n_tab + 1, 1], F32, kind="ExternalInput")
    rowptr_own_d = nc.dram_tensor("rowptr_own", [n_own + 1, 1], F32, kind="ExternalInput")
    gidx_d = nc.dram_tensor("gidx", [max(g_tot, 1), P, GATHER // 16], I16, kind="ExternalInput")
    gdstl_d = nc.dram_tensor("gdstl", [max(g_tot, 1), P, gpc], F32, kind="ExternalInput")
    nphant_d = nc.dram_tensor("nphant", [P, 1], F32, kind="ExternalInput")
    w_d = nc.dram_tensor("W", [D, D], F32, kind="ExternalInput")
    b_d = nc.dram_tensor("b", [D, 1], F32, kind="ExternalInput")
    gamma_d = nc.dram_tensor("gamma", [D, 1], F32, kind="ExternalInput")
    beta_d = nc.dram_tensor("beta", [D, 1], F32, kind="ExternalInput")

    out_d = nc.dram_tensor("out", [tpc, D, P], F32, kind="ExternalOutput")

    dinv_full_d = nc.dram_tensor("dinv_full", [n_tab, 1], F32)
    dinv_own_d = nc.dram_tensor("dinv_own", [1, n_own], F32)
    xto_d = nc.dram_tensor("xto", [tpc, D, P], F32)
    cc_in_d = nc.dram_tensor("cc_in", [P, 2], F32)
    cc_out_d = nc.dram_tensor("cc_out", [P, 2], F32)

    inv_n = 1.0 / float(n_nodes)

    with tile.TileContext(nc) as tc:
        with (
            tc.tile_pool(name="persist", bufs=1) as pp,
            tc.tile_pool(name="setup", bufs=1) as sp,
        ):
            # ---------- persistent tiles ----------
            a_big = pp.tile([P, n_own], dtype=F32)
            nc.vector.memset(a_big[:], 0)
            iota_i = sp.tile([P, P], dtype=mybir.dt.int32)
            nc.gpsimd.iota(iota_i[:], pattern=[[1, P]], base=0, channel_multiplier=0)
            iota_f = pp.tile([P, P], dtype=F32)
            nc.vector.tensor_copy(iota_f[:], iota_i[:])
            ident = pp.tile([P, P], dtype=F32)
            make_identity(nc, ident[:])
            w_sb = pp.tile([D, D], dtype=F32)
            nc.sync.dma_start(out=w_sb[:], in_=w_d[:])
            b_sb = pp.tile([D, 1], dtype=F32)
            nc.sync.dma_start(out=b_sb[:], in_=b_d[:])
            nb_sb = pp.tile([D, 1], dtype=F32)
            nc.vector.tensor_scalar(nb_sb[:], b_sb[:], -1.0, None, mybir.AluOpType.mult)
            gamma_sb = pp.tile([D, 1], dtype=F32)
            nc.sync.dma_start(out=gamma_sb[:], in_=gamma_d[:])
            beta_sb = pp.tile([D, 1], dtype=F32)
            nc.sync.dma_start(out=beta_sb[:], in_=beta_d[:])
            nph_sb = pp.tile([P, 1], dtype=F32)
            nc.sync.dma_start(out=nph_sb[:], in_=nphant_d[:])
            spos = pp.tile([P, tpc], dtype=F32)
            sneg = pp.tile([P, tpc], dtype=F32)
            qpos = pp.tile([P, tpc], dtype=F32)
            qneg = pp.tile([P, tpc], dtype=F32)

            # ---------- prologue A: dinv tables ----------
            rp0 = sp.tile([P, ncol], dtype=F32)
            rp1 = sp.tile([P, ncol], dtype=F32)
            rpf = rowptr_d[:].flatten()
            nc.sync.dma_start(out=rp0[:], in_=rpf[0:n_tab].rearrange("(p c) -> p c", p=P))
            nc.sync.dma_start(out=rp1[:], in_=rpf[1 : n_tab + 1].rearrange("(p c) -> p c", p=P))
            degm = sp.tile([P, ncol], dtype=F32)
            nc.vector.tensor_tensor(out=degm[:], in0=rp1[:], in1=rp0[:], op=mybir.AluOpType.subtract)
            sq = sp.tile([P, ncol], dtype=F32)
            nc.scalar.activation(sq[:], degm[:], mybir.ActivationFunctionType.Sqrt, bias=1.0)
            dinv_sb = sp.tile([P, ncol], dtype=F32)
            nc.vector.reciprocal(dinv_sb[:], sq[:])
            nc.sync.dma_start(
                out=dinv_full_d[:].flatten()[0:n_tab].rearrange("(p c) -> p c", p=P),
                in_=dinv_sb[:],
            )
            ro0 = sp.tile([P, tpc], dtype=F32)
            ro1 = sp.tile([P, tpc], dtype=F32)
            rof = rowptr_own_d[:].flatten()
            nc.sync.dma_start(out=ro0[:], in_=rof[0:n_own].rearrange("(p c) -> p c", p=P))
            nc.sync.dma_start(out=ro1[:], in_=rof[1 : n_own + 1].rearrange("(p c) -> p c", p=P))
            degmo = sp.tile([P, tpc], dtype=F32)
            nc.vector.tensor_tensor(out=degmo[:], in0=ro1[:], in1=ro0[:], op=mybir.AluOpType.subtract)
            sqo = sp.tile([P, tpc], dtype=F32)
            nc.scalar.activation(sqo[:], degmo[:], mybir.ActivationFunctionType.Sqrt, bias=1.0)
            dinvo_sb = pp.tile([P, tpc], dtype=F32)
            nc.vector.reciprocal(dinvo_sb[:], sqo[:])
            nc.sync.dma_start(
                out=dinv_own_d[:].flatten().rearrange("(p c) -> p c", p=P),
                in_=dinvo_sb[:],
            )

            # ---------- prologue C: x_own scaled+transposed tiles (self term) ----------
            with (
                tc.tile_pool(name="pc_in", bufs=3) as pcin,
                tc.tile_pool(name="pc_tr", bufs=3) as pctr,
                tc.tile_pool(name="pc_ps", bufs=3, space="PSUM") as pcps,
            ):
                # per-tile dinv_own as [128,1]: dinvo_sb is laid out (p, c) = node p*tpc+c,
                # which does NOT match tile-slicing; reload per tile from dinv_own_d.
                for t in range(tpc):
                    xo = pcin.tile([P, D], dtype=F32)
                    nc.sync.dma_start(out=xo[:], in_=xown_d[t * P : (t + 1) * P, :])
                    dv = pcin.tile([P, 1], dtype=F32)
                    nc.sync.dma_start(
                        out=dv[:], in_=dinv_own_d[:].flatten()[t * P : (t + 1) * P][:, None]
                    )
                    xs = pcin.tile([P, D], dtype=F32)
                    nc.gpsimd.tensor_scalar(xs[:], xo[:], dv[:], None, mybir.AluOpType.mult)
                    xtp = pcps.tile([P, P], dtype=F32, space="PSUM")
                    nc.tensor.transpose(out=xtp[:], in_=xs[:], identity=ident[:])
                    xt = pctr.tile([P, P], dtype=F32)
                    nc.scalar.activation(xt[:], xtp[:], mybir.ActivationFunctionType.Copy)
                    nc.sync.dma_start(out=xto_d[t], in_=xt[:])

            # ---------- main: slabs ----------
            goff = 0
            with (
                tc.tile_pool(name="slab", bufs=1) as slabp,
                tc.tile_pool(name="fill", bufs=4) as fillp,
                tc.tile_pool(name="frag", bufs=2) as fragp,
                tc.tile_pool(name="gmeta", bufs=2) as gmeta,
                tc.tile_pool(name="rps", bufs=4) as rpool,
                tc.tile_pool(name="trs", bufs=4) as trpool,
                tc.tile_pool(name="pst", bufs=3, space="PSUM") as pst,
                tc.tile_pool(name="fps", bufs=2, space="PSUM") as fps,
                tc.tile_pool(name="psc", bufs=2, space="PSUM") as psc,
            ):
                slab_sb = slabp.tile([P, slab_n[0]], dtype=F32)
                for s in range(S):
                    base = sum(slab_n[:s])
                    width = slab_n[s]
                    ntile = width // P
                    # fill slab: scale + XBAR-transpose x node-tiles into slab
                    for j in range(ntile):
                        g = base + j * P
                        xin = fillp.tile([P, D], dtype=F32)
                        nc.sync.dma_start(out=xin[:], in_=x_d[g : g + P, :])
                        dv = fillp.tile([P, 1], dtype=F32)
                        nc.sync.dma_start(
                            out=dv[:], in_=dinv_full_d[:].flatten()[g : g + P][:, None]
                        )
                        xs = fillp.tile([P, D], dtype=F32)
                        nc.gpsimd.tensor_scalar(xs[:], xin[:], dv[:], None, mybir.AluOpType.mult)
                        fpp = fps.tile([P, P], dtype=F32, space="PSUM")
                        nc.tensor.transpose(out=fpp[:], in_=xs[:], identity=ident[:])
                        nc.scalar.activation(
                            slab_sb[:, j * P : (j + 1) * P], fpp[:],
                            mybir.ActivationFunctionType.Copy,
                        )
                    # gather + chunk pipeline
                    chunks = sched[s]
                    cell_ps = None
                    for g in range(groups[s]):
                        gi = gmeta.tile([P, GATHER // 16], dtype=I16)
                        nc.sync.dma_start(out=gi[:], in_=gidx_d[goff + g])
                        gd = gmeta.tile([P, gpc], dtype=F32)
                        nc.sync.dma_start(out=gd[:], in_=gdstl_d[goff + g])
                        frag = fragp.tile([P, GATHER], dtype=F32)
                        nc.gpsimd.ap_gather(
                            out_ap=frag[:],
                            in_ap=slab_sb[:, 0:width],
                            idxs_ap=gi[:],
                            channels=P,
                            num_elems=width,
                            d=1,
                            num_idxs=GATHER,
                        )
                        for j in range(gpc):
                            ci = g * gpc + j
                            if ci >= len(chunks):
                                break
                            t, first, last = chunks[ci]
                            trp = pst.tile([P, P], dtype=F32, space="PSUM")
                            nc.tensor.transpose(
                                out=trp[:], in_=frag[:, j * P : (j + 1) * P], identity=ident[:]
                            )
                            tr = trpool.tile([P, P], dtype=F32)
                            nc.scalar.activation(tr[:], trp[:], mybir.ActivationFunctionType.Copy)
                            r_sb = rpool.tile([P, P], dtype=F32)
                            nc.vector.tensor_scalar(
                                r_sb[:], iota_f[:], gd[:, j : j + 1], None, mybir.AluOpType.is_equal
                            )
                            if first:
                                cell_ps = psc.tile([P, P], dtype=F32, space="PSUM")
                            nc.tensor.matmul(
                                out=cell_ps[:], lhsT=tr[:], rhs=r_sb[:], start=first, stop=last
                            )
                            if last:
                                blk = slice(t * P, (t + 1) * P)
                                nc.vector.tensor_tensor(
                                    out=a_big[:, blk], in0=a_big[:, blk], in1=cell_ps[:],
                                    op=mybir.AluOpType.add,
                                )
                    goff += groups[s]

            # ---------- phase 3: per-tile self + dinv_dst + W + bias + leaky ----------
            with (
                tc.tile_pool(name="dinvb", bufs=1) as dbp,
                tc.tile_pool(name="outb", bufs=1) as obp,
                tc.tile_pool(name="p3", bufs=3) as p3,
                tc.tile_pool(name="ps3", bufs=2, space="PSUM") as ps3,
            ):
                dinvb = dbp.tile([P, n_own], dtype=F32)
                nc.sync.dma_start(out=dinvb[:], in_=dinv_own_d[0:1, :].to_broadcast([P, n_own]))
                out_big = obp.tile([P, n_own], dtype=F32)
                for t in range(tpc):
                    blk = slice(t * P, (t + 1) * P)
                    xo = p3.tile([P, P], dtype=F32)
                    nc.sync.dma_start(out=xo[:], in_=xto_d[t])
                    at = p3.tile([P, P], dtype=F32)
                    nc.vector.tensor_tensor(
                        out=at[:], in0=a_big[:, blk], in1=xo[:], op=mybir.AluOpType.add
                    )
                    nc.vector.tensor_tensor(
                        out=at[:], in0=at[:], in1=dinvb[:, blk], op=mybir.AluOpType.mult
                    )
                    pc3 = ps3.tile([P, P], dtype=F32, space="PSUM")
                    nc.tensor.matmul(out=pc3[:], lhsT=w_sb[:], rhs=at[:], start=True, stop=True)
                    pos = p3.tile([P, P], dtype=F32)
                    neg = p3.tile([P, P], dtype=F32)
                    scr = p3.tile([P, P], dtype=F32)
                    scr2 = p3.tile([P, P], dtype=F32)
                    nc.scalar.activation(
                        pos[:], pc3[:], mybir.ActivationFunctionType.Relu,
                        bias=b_sb[:], scale=1.0, accum_out=spos[:, t : t + 1],
                    )
                    nc.scalar.activation(
                        neg[:], pc3[:], mybir.ActivationFunctionType.Relu,
                        bias=nb_sb[:], scale=-1.0, accum_out=sneg[:, t : t + 1],
                    )
                    nc.scalar.activation(
                        scr[:], pos[:], mybir.ActivationFunctionType.Square,
                        accum_out=qpos[:, t : t + 1],
                    )
                    nc.scalar.activation(
                        scr2[:], neg[:], mybir.ActivationFunctionType.Square,
                        accum_out=qneg[:, t : t + 1],
                    )
                    nc.gpsimd.tensor_scalar(neg[:], neg[:], LEAKY, None, mybir.AluOpType.mult)
                    nc.gpsimd.tensor_tensor(
                        out=out_big[:, blk], in0=pos[:], in1=neg[:], op=mybir.AluOpType.subtract
                    )

                # ---------- phase 4: BN stats + AllReduce + affine + store ----------
                rsp = sp.tile([P, 1], dtype=F32)
                rsn = sp.tile([P, 1], dtype=F32)
                rqp = sp.tile([P, 1], dtype=F32)
                rqn = sp.tile([P, 1], dtype=F32)
                nc.vector.tensor_reduce(rsp[:], spos[:], mybir.AxisListType.X, mybir.AluOpType.add)
                nc.vector.tensor_reduce(rsn[:], sneg[:], mybir.AxisListType.X, mybir.AluOpType.add)
                nc.vector.tensor_reduce(rqp[:], qpos[:], mybir.AxisListType.X, mybir.AluOpType.add)
                nc.vector.tensor_reduce(rqn[:], qneg[:], mybir.AxisListType.X, mybir.AluOpType.add)
                s_lr = sp.tile([P, 1], dtype=F32)
                nc.vector.tensor_scalar(s_lr[:], rsn[:], -LEAKY, None, mybir.AluOpType.mult)
                nc.vector.tensor_tensor(out=s_lr[:], in0=s_lr[:], in1=rsp[:], op=mybir.AluOpType.add)
                q_lr = sp.tile([P, 1], dtype=F32)
                nc.vector.tensor_scalar(q_lr[:], rqn[:], LEAKY * LEAKY, None, mybir.AluOpType.mult)
                nc.vector.tensor_tensor(out=q_lr[:], in0=q_lr[:], in1=rqp[:], op=mybir.AluOpType.add)
                pb = sp.tile([P, 1], dtype=F32)
                nb2 = sp.tile([P, 1], dtype=F32)
                nc.scalar.activation(pb[:], b_sb[:], mybir.ActivationFunctionType.Relu)
                nc.scalar.activation(nb2[:], b_sb[:], mybir.ActivationFunctionType.Relu, scale=-1.0)
                lb = sp.tile([P, 1], dtype=F32)
                nc.vector.tensor_scalar(lb[:], nb2[:], -LEAKY, None, mybir.AluOpType.mult)
                nc.vector.tensor_tensor(out=lb[:], in0=lb[:], in1=pb[:], op=mybir.AluOpType.add)
                lb2 = sp.tile([P, 1], dtype=F32)
                nc.scalar.activation(lb2[:], lb[:], mybir.ActivationFunctionType.Square)
                corr = sp.tile([P, 1], dtype=F32)
                nc.vector.tensor_tensor(out=corr[:], in0=nph_sb[:], in1=lb[:], op=mybir.AluOpType.mult)
                nc.vector.tensor_tensor(out=s_lr[:], in0=s_lr[:], in1=corr[:], op=mybir.AluOpType.subtract)
                nc.vector.tensor_tensor(out=corr[:], in0=nph_sb[:], in1=lb2[:], op=mybir.AluOpType.mult)
                nc.vector.tensor_tensor(out=q_lr[:], in0=q_lr[:], in1=corr[:], op=mybir.AluOpType.subtract)

                cc_sb = sp.tile([P, 2], dtype=F32)
                nc.vector.tensor_copy(cc_sb[:, 0:1], s_lr[:])
                nc.vector.tensor_copy(cc_sb[:, 1:2], q_lr[:])
                nc.sync.dma_start(out=cc_in_d[:], in_=cc_sb[:])
                nc.gpsimd.collective_compute(
                    "AllReduce",
                    mybir.AluOpType.add,
                    replica_groups=[list(range(N_CORES))],
                    ins=[cc_in_d[:]],
                    outs=[cc_out_d[:]],
                )
                st = sp.tile([P, 2], dtype=F32)
                nc.sync.dma_start(out=st[:], in_=cc_out_d[:])
                mean = sp.tile([P, 1], dtype=F32)
                nc.vector.tensor_scalar(mean[:], st[:, 0:1], inv_n, None, mybir.AluOpType.mult)
                msq = sp.tile([P, 1], dtype=F32)
                nc.vector.tensor_scalar(msq[:], st[:, 1:2], inv_n, None, mybir.AluOpType.mult)
                m2 = sp.tile([P, 1], dtype=F32)
                nc.scalar.activation(m2[:], mean[:], mybir.ActivationFunctionType.Square)
                var = sp.tile([P, 1], dtype=F32)
                nc.vector.tensor_tensor(out=var[:], in0=msq[:], in1=m2[:], op=mybir.AluOpType.subtract)
                nc.vector.tensor_scalar(var[:], var[:], BN_EPS, None, mybir.AluOpType.add)
                sd = sp.tile([P, 1], dtype=F32)
                nc.scalar.activation(sd[:], var[:], mybir.ActivationFunctionType.Sqrt)
                rstd = sp.tile([P, 1], dtype=F32)
                nc.vector.reciprocal(rstd[:], sd[:])
                sfac = sp.tile([P, 1], dtype=F32)
                nc.vector.tensor_tensor(out=sfac[:], in0=gamma_sb[:], in1=rstd[:], op=mybir.AluOpType.mult)
                tsh = sp.tile([P, 1], dtype=F32)
                nc.vector.tensor_tensor(out=tsh[:], in0=mean[:], in1=sfac[:], op=mybir.AluOpType.mult)
                nc.vector.tensor_tensor(out=tsh[:], in0=beta_sb[:], in1=tsh[:], op=mybir.AluOpType.subtract)

                for t in range(tpc):
                    blk = slice(t * P, (t + 1) * P)
                    fin = p3.tile([P, P], dtype=F32)
                    nc.scalar.activation(
                        fin[:], out_big[:, blk], mybir.ActivationFunctionType.Identity,
                        bias=tsh[:], scale=sfac[:],
                    )
                    nc.sync.dma_start(out=out_d[t], in_=fin[:])

    nc.compile()
    return nc


def _prep(x, edge_index, n_nodes, tpc, slab_target=25088):
    """Host-side sharding: bucket edges by (src slab, dst tile), pad, wrap."""
    n_tiles = N_CORES * tpc
    n_pad = n_tiles * P
    n_tab = ((n_nodes + P - 1) // P) * P
    n_own = tpc * P

    # slab partition of the source-node table
    S = max(1, -(-n_tab // slab_target))
    slab_n = []
    rem = n_tab
    for s in range(S):
        w = min(slab_target, rem)
        slab_n.append(w)
        rem -= w
    assert sum(slab_n) == n_tab
    slab_starts = np.cumsum([0] + slab_n)

    src = np.ascontiguousarray(edge_index[0]).astype(np.int64)
    dst = np.ascontiguousarray(edge_index[1]).astype(np.int64)

    # degree / rowptr over all real edges
    counts_nodes = np.bincount(dst, minlength=n_pad)
    rowptr = np.zeros(n_pad + 1, np.int64)
    np.cumsum(counts_nodes, out=rowptr[1:])
    rowptr_f = rowptr[: n_tab + 1].astype(np.float32)

    core_of = dst // n_own
    slab_of = np.searchsorted(slab_starts, src, side="right") - 1
    tile_of = (dst % n_own) // P

    # per-core per-cell counts -> max over cores
    cell_id = (core_of * S + slab_of) * tpc + tile_of
    cc = np.bincount(cell_id, minlength=N_CORES * S * tpc).reshape(N_CORES, S, tpc)
    counts_max = cc.max(axis=0)
    k_cell, sched, groups = _plan(counts_max, S, tpc)

    gpc = GATHER // CHUNK
    g_tot = sum(groups)

    # per-core edge data, ordered by (slab, tile)
    in_maps = []
    order = np.lexsort((tile_of, slab_of, core_of))
    src_s = src[order]
    dst_s = dst[order]
    slab_s = slab_of[order]
    core_s = core_of[order]
    tile_s = tile_of[order]
    core_bounds = np.searchsorted(core_s, np.arange(N_CORES + 1))

    x_pad = x
    if n_tab > n_nodes:
        x_pad = np.concatenate([x, np.zeros((n_tab - n_nodes, D), np.float32)], 0)

    for c in range(N_CORES):
        lo_e, hi_e = core_bounds[c], core_bounds[c + 1]
        csrc = src_s[lo_e:hi_e]
        cdst = dst_s[lo_e:hi_e]
        cslab = slab_s[lo_e:hi_e]
        ctile = tile_s[lo_e:hi_e]
        lo = c * n_own

        gidx = np.zeros((max(g_tot, 1), P, GATHER // 16), np.int16)
        gdstl = np.full((max(g_tot, 1), P, gpc), -1.0, np.float32)
        goff = 0
        # boundaries of (slab, tile) runs in this core's sorted edge list
        for s in range(S):
            chunks = sched[s]
            n_chunks = len(chunks)
            idx_stream = np.zeros(max(n_chunks, 1) * CHUNK, np.int16)
            dstl_stream = np.full(max(n_chunks, 1) * CHUNK, -1.0, np.float32)
            sm = cslab == s
            ssrc = csrc[sm] - slab_starts[s]
            sdst = cdst[sm]
            stile = ctile[sm]
            pos = 0
            ei = 0
            for t, first, last in chunks:
                if first:
                    # locate this cell's edges
                    cellm = stile == t
                    cell_src = ssrc[cellm]
                    cell_dl = (sdst[cellm] % P).astype(np.float32)
                    ei = 0
                n_take = max(0, min(CHUNK, len(cell_src) - ei))
                if n_take > 0:
                    idx_stream[pos : pos + n_take] = cell_src[ei : ei + n_take]
                    dstl_stream[pos : pos + n_take] = cell_dl[ei : ei + n_take]
                    ei += n_take
                pos += CHUNK
            # pack into gather groups
            for g in range(groups[s]):
                seg = idx_stream[g * GATHER : (g + 1) * GATHER]
                buf = np.zeros(GATHER, np.int16)
                buf[: len(seg)] = seg
                wrapped = buf.reshape(GATHER // 16, 16).T  # [16, GATHER//16]
                gidx[goff + g] = np.tile(wrapped, (8, 1))
                dseg = dstl_stream[g * GATHER : (g + 1) * GATHER]
                dbuf = np.full(GATHER, -1.0, np.float32)
                dbuf[: len(dseg)] = dseg
                gdstl[goff + g] = dbuf.reshape(gpc, CHUNK).T  # [128, gpc]
            goff += groups[s]

        nph = float(max(0, min(lo + n_own, n_pad) - max(lo, n_nodes)))
        xo = x_pad[lo : lo + n_own] if lo + n_own <= n_tab else np.concatenate(
            [x_pad[lo:], np.zeros((lo + n_own - n_tab, D), np.float32)], 0
        )
        rpo = rowptr[lo : lo + n_own + 1].astype(np.float32)
        in_maps.append(
            {
                "x": x_pad,
                "x_own": np.ascontiguousarray(xo),
                "rowptr": rowptr_f.reshape(-1, 1),
                "rowptr_own": rpo.reshape(-1, 1),
                "gidx": gidx,
                "gdstl": gdstl,
                "nphant": np.full((P, 1), nph, np.float32),
            }
        )
    return in_maps, slab_n, sched, groups


def _run(x, edge_index, W, b, gamma, beta, n_nodes, tpc, trace=False, tmpdir=None,
         slab_target=25088, sim=False):
    global LAST_RESULTS
    x = np.ascontiguousarray(x, np.float32)
    in_maps, slab_n, sched, groups = _prep(x, edge_index, n_nodes, tpc, slab_target)
    for m in in_maps:
        m["W"] = np.ascontiguousarray(W, np.float32)
        m["b"] = np.ascontiguousarray(b, np.float32).reshape(D, 1)
        m["gamma"] = np.ascontiguousarray(gamma, np.float32).reshape(D, 1)
        m["beta"] = np.ascontiguousarray(beta, np.float32).reshape(D, 1)
    nc = _build_program(n_nodes, tpc, slab_n, sched, groups)
    if sim:
        from concourse import bass_interp

        msim = bass_interp.MultiCoreSim(nc, N_CORES)
        for c in range(N_CORES):
            for k, v in in_maps[c].items():
                msim.cores[c].tensor(k)[:] = v
        msim.simulate()
        results = [{"out": np.asarray(msim.cores[c].tensor("out"))} for c in range(N_CORES)]
        LAST_RESULTS = None
    else:
        res = run_bass_kernel_spmd(
            nc, in_maps, list(range(N_CORES)), trace=trace, tmpdir=tmpdir
        )
        LAST_RESULTS = res
        results = res.results
    blocks = [
        results[c]["out"].transpose(0, 2, 1).reshape(tpc * P, D) for c in range(N_CORES)
    ]
    return np.concatenate(blocks, axis=0)[:n_nodes]


def kernel(x, edge_index, W, b, gamma, beta):
    x = np.ascontiguousarray(x, np.float32)
    n_nodes = x.shape[0]
    tpc = (n_nodes + N_CORES * P - 1) // (N_CORES * P)
    out = _run(x, edge_index, W, b, gamma, beta, n_nodes, tpc)
    return out.astype(np.float32)


# revision 9
# speedup vs baseline: 1.3097x; 1.3097x over previous
"""GCN conv block (gather -> normalized scatter-add -> matmul -> bias ->
LeakyReLU -> BatchNorm) on 8 Trainium2 NeuronCores.

v2 architecture (per core; SPMD single program, nodes sharded by range):
  - Prologue: dinv = 1/sqrt(1+in_degree) for all nodes from rowptr (device);
    per-core own-range dinv likewise.
  - Main loop over S source slabs. Slab fill: stream x node-tiles, scale by
    dinv (gpsimd), DMA-XBAR-transpose (2x 64-partition halves) into a resident
    f-major slab [128 f, SLAB nodes] in SBUF. Edge gathers then run as big
    gpsimd ap_gather ops (2048 edges each) producing f-major fragments; each
    128-edge chunk is PE-transposed to edge-major and multiplied with an
    on-the-fly one-hot R [e,d] = (dst_local==d) to segment-sum into per-tile
    PSUM, accumulated across slabs in an SBUF A_big [128 f, 12544 d].
  - Phase 3: per destination tile, add the self-loop term (dinv^2 x built via
    the same scale+transpose path from per-core x_own), scale columns by
    dinv[dst] (broadcast tile), apply W, bias, LeakyReLU (relu(z)-0.01relu(-z)
    with fused BN sum/sumsq accumulators).
  - Phase 4: AllReduce BN stats across the 8 cores, apply the affine, store
    out^T per tile. Host reassembles.
Host only shards/pads index structures (CSR bucketing) and reassembles.
"""
import sys

for _p in ("/opt/trn_rl_repo",):
    if _p not in sys.path:
        sys.path.insert(0, _p)

import numpy as np

from concourse import bass, bacc, mybir
import concourse.tile as tile
from concourse.bass_utils import run_bass_kernel_spmd
from concourse.masks import make_identity

P = 128
D = 128
N_CORES = 8
LEAKY = 0.01
BN_EPS = 1e-5
CHUNK = 128          # edges per matmul chunk
GATHER = 2048        # edges per ap_gather instruction
F32 = mybir.dt.float32
I16 = mybir.dt.int16

LAST_RESULTS = None


def _plan(counts_max, S, tpc):
    """counts_max: [S, tpc] max-over-cores edge counts per (slab, tile) cell.
    Returns per-slab chunk schedules: list (per s) of list of (tile, first, last),
    and group counts per slab."""
    k_cell = -(-counts_max // CHUNK)  # ceil; zeros stay zero
    sched = []
    groups = []
    for s in range(S):
        chunks = []
        for t in range(tpc):
            k = int(k_cell[s, t])
            for j in range(k):
                chunks.append((t, j == 0, j == k - 1))
        sched.append(chunks)
        groups.append(-(-len(chunks) // (GATHER // CHUNK)))
    return k_cell, sched, groups


def _build_program(n_nodes, tpc, slab_n, sched, groups):
    n_tab = ((n_nodes + P - 1) // P) * P
    ncol = n_tab // P
    n_own = tpc * P
    S = len(sched)
    g_tot = sum(groups)
    gpc = GATHER // CHUNK  # chunks per gather group

    nc = bacc.Bacc("TRN2", target_bir_lowering=False, debug=False)

    x_d = nc.dram_tensor("x", [n_tab, D], F32, kind="ExternalInput")
    xown_d = nc.dram_tensor("x_own", [n_own, D], F32, kind="ExternalInput")
    rowptr_d = nc.dram_tensor("rowptr", [n_tab + 1, 1], F32, kind="ExternalInput")
    rowptr_own_d = nc.dram_tensor("rowptr_own", [n_own + 1, 1], F32, kind="ExternalInput")
    gidx_d = nc.dram_tensor("gidx", [max(g_tot, 1), P, GATHER // 16], I16, kind="ExternalInput")
    gdstl_d = nc.dram_tensor("gdstl", [max(g_tot, 1), P, gpc], F32, kind="ExternalInput")
    nphant_d = nc.dram_tensor("nphant", [P, 1], F32, kind="ExternalInput")
    w_d = nc.dram_tensor("W", [D, D], F32, kind="ExternalInput")
    b_d = nc.dram_tensor("b", [D, 1], F32, kind="ExternalInput")
    gamma_d = nc.dram_tensor("gamma", [D, 1], F32, kind="ExternalInput")
    beta_d = nc.dram_tensor("beta", [D, 1], F32, kind="ExternalInput")

    out_d = nc.dram_tensor("out", [tpc, D, P], F32, kind="ExternalOutput")

    dinv_full_d = nc.dram_tensor("dinv_full", [n_tab, 1], F32)
    dinv_own_d = nc.dram_tensor("dinv_own", [1, n_own], F32)
    xto_d = nc.dram_tensor("xto", [tpc, D, P], F32)
    cc_in_d = nc.dram_tensor("cc_in", [P, 2], F32)
    cc_out_d = nc.dram_tensor("cc_out", [P, 2], F32)

    inv_n = 1.0 / float(n_nodes)

    with tile.TileContext(nc) as tc:
        with (
            tc.tile_pool(name="persist", bufs=1) as pp,
            tc.tile_pool(name="setup", bufs=1) as sp,
        ):
            # ---------- persistent tiles ----------
            a_big = pp.tile([P, n_own], dtype=F32)
            nc.vector.memset(a_big[:], 0)
            iota_i = sp.tile([P, P], dtype=mybir.dt.int32)
            nc.gpsimd.iota(iota_i[:], pattern=[[1, P]], base=0, channel_multiplier=0)
            iota_f = pp.tile([P, P], dtype=F32)
            nc.vector.tensor_copy(iota_f[:], iota_i[:])
            ident = pp.tile([P, P], dtype=F32)
            make_identity(nc, ident[:])
            w_sb = pp.tile([D, D], dtype=F32)
            nc.sync.dma_start(out=w_sb[:], in_=w_d[:])
            b_sb = pp.tile([D, 1], dtype=F32)
            nc.sync.dma_start(out=b_sb[:], in_=b_d[:])
            nb_sb = pp.tile([D, 1], dtype=F32)
            nc.vector.tensor_scalar(nb_sb[:], b_sb[:], -1.0, None, mybir.AluOpType.mult)
            gamma_sb = pp.tile([D, 1], dtype=F32)
            nc.sync.dma_start(out=gamma_sb[:], in_=gamma_d[:])
            beta_sb = pp.tile([D, 1], dtype=F32)
            nc.sync.dma_start(out=beta_sb[:], in_=beta_d[:])
            nph_sb = pp.tile([P, 1], dtype=F32)
            nc.sync.dma_start(out=nph_sb[:], in_=nphant_d[:])
            spos = pp.tile([P, tpc], dtype=F32)
            sneg = pp.tile([P, tpc], dtype=F32)
            qpos = pp.tile([P, tpc], dtype=F32)
            qneg = pp.tile([P, tpc], dtype=F32)

            # ---------- prologue A: dinv tables ----------
            rp0 = sp.tile([P, ncol], dtype=F32)
            rp1 = sp.tile([P, ncol], dtype=F32)
            rpf = rowptr_d[:].flatten()
            nc.sync.dma_start(out=rp0[:], in_=rpf[0:n_tab].rearrange("(p c) -> p c", p=P))
            nc.sync.dma_start(out=rp1[:], in_=rpf[1 : n_tab + 1].rearrange("(p c) -> p c", p=P))
            degm = sp.tile([P, ncol], dtype=F32)
            nc.vector.tensor_tensor(out=degm[:], in0=rp1[:], in1=rp0[:], op=mybir.AluOpType.subtract)
            sq = sp.tile([P, ncol], dtype=F32)
            nc.scalar.activation(sq[:], degm[:], mybir.ActivationFunctionType.Sqrt, bias=1.0)
            dinv_sb = sp.tile([P, ncol], dtype=F32)
            nc.vector.reciprocal(dinv_sb[:], sq[:])
            nc.sync.dma_start(
                out=dinv_full_d[:].flatten()[0:n_tab].rearrange("(p c) -> p c", p=P),
                in_=dinv_sb[:],
            )
            ro0 = sp.tile([P, tpc], dtype=F32)
            ro1 = sp.tile([P, tpc], dtype=F32)
            rof = rowptr_own_d[:].flatten()
            nc.sync.dma_start(out=ro0[:], in_=rof[0:n_own].rearrange("(p c) -> p c", p=P))
            nc.sync.dma_start(out=ro1[:], in_=rof[1 : n_own + 1].rearrange("(p c) -> p c", p=P))
            degmo = sp.tile([P, tpc], dtype=F32)
            nc.vector.tensor_tensor(out=degmo[:], in0=ro1[:], in1=ro0[:], op=mybir.AluOpType.subtract)
            sqo = sp.tile([P, tpc], dtype=F32)
            nc.scalar.activation(sqo[:], degmo[:], mybir.ActivationFunctionType.Sqrt, bias=1.0)
            dinvo_sb = pp.tile([P, tpc], dtype=F32)
            nc.vector.reciprocal(dinvo_sb[:], sqo[:])
            nc.sync.dma_start(
                out=dinv_own_d[:].flatten().rearrange("(p c) -> p c", p=P),
                in_=dinvo_sb[:],
            )

            # ---------- prologue C: x_own scaled+transposed tiles (self term) ----------
            with (
                tc.tile_pool(name="pc_in", bufs=3) as pcin,
                tc.tile_pool(name="pc_tr", bufs=3) as pctr,
                tc.tile_pool(name="pc_ps", bufs=3, space="PSUM") as pcps,
            ):
                # per-tile dinv_own as [128,1]: dinvo_sb is laid out (p, c) = node p*tpc+c,
                # which does NOT match tile-slicing; reload per tile from dinv_own_d.
                for t in range(tpc):
                    xo = pcin.tile([P, D], dtype=F32)
                    nc.sync.dma_start(out=xo[:], in_=xown_d[t * P : (t + 1) * P, :])
                    dv = pcin.tile([P, 1], dtype=F32)
                    nc.sync.dma_start(
                        out=dv[:], in_=dinv_own_d[:].flatten()[t * P : (t + 1) * P][:, None]
                    )
                    xs = pcin.tile([P, D], dtype=F32)
                    nc.vector.tensor_scalar(xs[:], xo[:], dv[:], None, mybir.AluOpType.mult)
                    xtp = pcps.tile([P, P], dtype=F32, space="PSUM")
                    nc.tensor.transpose(out=xtp[:], in_=xs[:], identity=ident[:])
                    xt = pctr.tile([P, P], dtype=F32)
                    nc.scalar.activation(xt[:], xtp[:], mybir.ActivationFunctionType.Copy)
                    nc.sync.dma_start(out=xto_d[t], in_=xt[:])

            # ---------- main: slabs ----------
            goff = 0
            with (
                tc.tile_pool(name="slab", bufs=1) as slabp,
                tc.tile_pool(name="fill", bufs=4) as fillp,
                tc.tile_pool(name="frag", bufs=2) as fragp,
                tc.tile_pool(name="gmeta", bufs=2) as gmeta,
                tc.tile_pool(name="rps", bufs=4) as rpool,
                tc.tile_pool(name="trs", bufs=4) as trpool,
                tc.tile_pool(name="pst", bufs=3, space="PSUM") as pst,
                tc.tile_pool(name="fps", bufs=2, space="PSUM") as fps,
                tc.tile_pool(name="psc", bufs=2, space="PSUM") as psc,
            ):
                slab_sb = slabp.tile([P, slab_n[0]], dtype=F32)
                for s in range(S):
                    base = sum(slab_n[:s])
                    width = slab_n[s]
                    ntile = width // P
                    # fill slab: scale + PE-transpose x node-tiles into slab
                    FB = 4
                    nblk = -(-ntile // FB)
                    for jb in range(nblk):
                        j0 = jb * FB
                        nt = min(FB, ntile - j0)
                        g = base + j0 * P
                        xin = fillp.tile([P, nt, D], dtype=F32)
                        nc.sync.dma_start(
                            out=xin[:],
                            in_=x_d[g : g + nt * P, :].rearrange("(k p) f -> p k f", p=P),
                        )
                        dv = fillp.tile([P, nt], dtype=F32)
                        nc.gpsimd.dma_start(
                            out=dv[:],
                            in_=dinv_full_d[:].flatten()[g : g + nt * P].rearrange(
                                "(k p) -> p k", p=P
                            ),
                        )
                        xs = fillp.tile([P, nt, D], dtype=F32)
                        nc.vector.tensor_tensor(
                            out=xs[:], in0=xin[:],
                            in1=dv[:, :, None].to_broadcast([P, nt, D]),
                            op=mybir.AluOpType.mult,
                        )
                        for k in range(nt):
                            j = j0 + k
                            fpp = fps.tile([P, P], dtype=F32, space="PSUM")
                            nc.tensor.transpose(out=fpp[:], in_=xs[:, k, :], identity=ident[:])
                            nc.scalar.activation(
                                slab_sb[:, j * P : (j + 1) * P], fpp[:],
                                mybir.ActivationFunctionType.Copy,
                            )
                    # gather + chunk pipeline
                    chunks = sched[s]
                    cell_ps = None
                    for g in range(groups[s]):
                        gi = gmeta.tile([P, GATHER // 16], dtype=I16)
                        nc.scalar.dma_start(out=gi[:], in_=gidx_d[goff + g])
                        gd = gmeta.tile([P, gpc], dtype=F32)
                        nc.scalar.dma_start(out=gd[:], in_=gdstl_d[goff + g])
                        frag = fragp.tile([P, GATHER], dtype=F32)
                        nc.gpsimd.ap_gather(
                            out_ap=frag[:],
                            in_ap=slab_sb[:, 0:width],
                            idxs_ap=gi[:],
                            channels=P,
                            num_elems=width,
                            d=1,
                            num_idxs=GATHER,
                        )
                        for j in range(gpc):
                            ci = g * gpc + j
                            if ci >= len(chunks):
                                break
                            t, first, last = chunks[ci]
                            trp = pst.tile([P, P], dtype=F32, space="PSUM")
                            nc.tensor.transpose(
                                out=trp[:], in_=frag[:, j * P : (j + 1) * P], identity=ident[:]
                            )
                            tr = trpool.tile([P, P], dtype=F32)
                            nc.vector.tensor_copy(tr[:], trp[:])
                            r_sb = rpool.tile([P, P], dtype=F32)
                            nc.vector.tensor_scalar(
                                r_sb[:], iota_f[:], gd[:, j : j + 1], None, mybir.AluOpType.is_equal
                            )
                            if first:
                                cell_ps = psc.tile([P, P], dtype=F32, space="PSUM")
                            nc.tensor.matmul(
                                out=cell_ps[:], lhsT=tr[:], rhs=r_sb[:], start=first, stop=last
                            )
                            if last:
                                blk = slice(t * P, (t + 1) * P)
                                nc.vector.tensor_tensor(
                                    out=a_big[:, blk], in0=a_big[:, blk], in1=cell_ps[:],
                                    op=mybir.AluOpType.add,
                                )
                    goff += groups[s]

            # ---------- phase 3: per-tile self + dinv_dst + W + bias + leaky ----------
            with (
                tc.tile_pool(name="dinvb", bufs=1) as dbp,
                tc.tile_pool(name="outb", bufs=1) as obp,
                tc.tile_pool(name="p3", bufs=3) as p3,
                tc.tile_pool(name="ps3", bufs=2, space="PSUM") as ps3,
            ):
                dinvb = dbp.tile([P, n_own], dtype=F32)
                nc.sync.dma_start(out=dinvb[:], in_=dinv_own_d[0:1, :].to_broadcast([P, n_own]))
                out_big = obp.tile([P, n_own], dtype=F32)
                for t in range(tpc):
                    blk = slice(t * P, (t + 1) * P)
                    xo = p3.tile([P, P], dtype=F32)
                    nc.sync.dma_start(out=xo[:], in_=xto_d[t])
                    at = p3.tile([P, P], dtype=F32)
                    nc.vector.tensor_tensor(
                        out=at[:], in0=a_big[:, blk], in1=xo[:], op=mybir.AluOpType.add
                    )
                    nc.vector.tensor_tensor(
                        out=at[:], in0=at[:], in1=dinvb[:, blk], op=mybir.AluOpType.mult
                    )
                    pc3 = ps3.tile([P, P], dtype=F32, space="PSUM")
                    nc.tensor.matmul(out=pc3[:], lhsT=w_sb[:], rhs=at[:], start=True, stop=True)
                    pos = p3.tile([P, P], dtype=F32)
                    neg = p3.tile([P, P], dtype=F32)
                    scr = p3.tile([P, P], dtype=F32)
                    scr2 = p3.tile([P, P], dtype=F32)
                    nc.scalar.activation(
                        pos[:], pc3[:], mybir.ActivationFunctionType.Relu,
                        bias=b_sb[:], scale=1.0, accum_out=spos[:, t : t + 1],
                    )
                    nc.scalar.activation(
                        neg[:], pc3[:], mybir.ActivationFunctionType.Relu,
                        bias=nb_sb[:], scale=-1.0, accum_out=sneg[:, t : t + 1],
                    )
                    nc.scalar.activation(
                        scr[:], pos[:], mybir.ActivationFunctionType.Square,
                        accum_out=qpos[:, t : t + 1],
                    )
                    nc.scalar.activation(
                        scr2[:], neg[:], mybir.ActivationFunctionType.Square,
                        accum_out=qneg[:, t : t + 1],
                    )
                    nc.gpsimd.tensor_scalar(neg[:], neg[:], LEAKY, None, mybir.AluOpType.mult)
                    nc.gpsimd.tensor_tensor(
                        out=out_big[:, blk], in0=pos[:], in1=neg[:], op=mybir.AluOpType.subtract
                    )

                # ---------- phase 4: BN stats + AllReduce + affine + store ----------
                rsp = sp.tile([P, 1], dtype=F32)
                rsn = sp.tile([P, 1], dtype=F32)
                rqp = sp.tile([P, 1], dtype=F32)
                rqn = sp.tile([P, 1], dtype=F32)
                nc.vector.tensor_reduce(rsp[:], spos[:], mybir.AxisListType.X, mybir.AluOpType.add)
                nc.vector.tensor_reduce(rsn[:], sneg[:], mybir.AxisListType.X, mybir.AluOpType.add)
                nc.vector.tensor_reduce(rqp[:], qpos[:], mybir.AxisListType.X, mybir.AluOpType.add)
                nc.vector.tensor_reduce(rqn[:], qneg[:], mybir.AxisListType.X, mybir.AluOpType.add)
                s_lr = sp.tile([P, 1], dtype=F32)
                nc.vector.tensor_scalar(s_lr[:], rsn[:], -LEAKY, None, mybir.AluOpType.mult)
                nc.vector.tensor_tensor(out=s_lr[:], in0=s_lr[:], in1=rsp[:], op=mybir.AluOpType.add)
                q_lr = sp.tile([P, 1], dtype=F32)
                nc.vector.tensor_scalar(q_lr[:], rqn[:], LEAKY * LEAKY, None, mybir.AluOpType.mult)
                nc.vector.tensor_tensor(out=q_lr[:], in0=q_lr[:], in1=rqp[:], op=mybir.AluOpType.add)
                pb = sp.tile([P, 1], dtype=F32)
                nb2 = sp.tile([P, 1], dtype=F32)
                nc.scalar.activation(pb[:], b_sb[:], mybir.ActivationFunctionType.Relu)
                nc.scalar.activation(nb2[:], b_sb[:], mybir.ActivationFunctionType.Relu, scale=-1.0)
                lb = sp.tile([P, 1], dtype=F32)
                nc.vector.tensor_scalar(lb[:], nb2[:], -LEAKY, None, mybir.AluOpType.mult)
                nc.vector.tensor_tensor(out=lb[:], in0=lb[:], in1=pb[:], op=mybir.AluOpType.add)
                lb2 = sp.tile([P, 1], dtype=F32)
                nc.scalar.activation(lb2[:], lb[:], mybir.ActivationFunctionType.Square)
                corr = sp.tile([P, 1], dtype=F32)
                nc.vector.tensor_tensor(out=corr[:], in0=nph_sb[:], in1=lb[:], op=mybir.AluOpType.mult)
                nc.vector.tensor_tensor(out=s_lr[:], in0=s_lr[:], in1=corr[:], op=mybir.AluOpType.subtract)
                nc.vector.tensor_tensor(out=corr[:], in0=nph_sb[:], in1=lb2[:], op=mybir.AluOpType.mult)
                nc.vector.tensor_tensor(out=q_lr[:], in0=q_lr[:], in1=corr[:], op=mybir.AluOpType.subtract)

                cc_sb = sp.tile([P, 2], dtype=F32)
                nc.vector.tensor_copy(cc_sb[:, 0:1], s_lr[:])
                nc.vector.tensor_copy(cc_sb[:, 1:2], q_lr[:])
                nc.sync.dma_start(out=cc_in_d[:], in_=cc_sb[:])
                nc.gpsimd.collective_compute(
                    "AllReduce",
                    mybir.AluOpType.add,
                    replica_groups=[list(range(N_CORES))],
                    ins=[cc_in_d[:]],
                    outs=[cc_out_d[:]],
                )
                st = sp.tile([P, 2], dtype=F32)
                nc.sync.dma_start(out=st[:], in_=cc_out_d[:])
                mean = sp.tile([P, 1], dtype=F32)
                nc.vector.tensor_scalar(mean[:], st[:, 0:1], inv_n, None, mybir.AluOpType.mult)
                msq = sp.tile([P, 1], dtype=F32)
                nc.vector.tensor_scalar(msq[:], st[:, 1:2], inv_n, None, mybir.AluOpType.mult)
                m2 = sp.tile([P, 1], dtype=F32)
                nc.scalar.activation(m2[:], mean[:], mybir.ActivationFunctionType.Square)
                var = sp.tile([P, 1], dtype=F32)
                nc.vector.tensor_tensor(out=var[:], in0=msq[:], in1=m2[:], op=mybir.AluOpType.subtract)
                nc.vector.tensor_scalar(var[:], var[:], BN_EPS, None, mybir.AluOpType.add)
                sd = sp.tile([P, 1], dtype=F32)
                nc.scalar.activation(sd[:], var[:], mybir.ActivationFunctionType.Sqrt)
                rstd = sp.tile([P, 1], dtype=F32)
                nc.vector.reciprocal(rstd[:], sd[:])
                sfac = sp.tile([P, 1], dtype=F32)
                nc.vector.tensor_tensor(out=sfac[:], in0=gamma_sb[:], in1=rstd[:], op=mybir.AluOpType.mult)
                tsh = sp.tile([P, 1], dtype=F32)
                nc.vector.tensor_tensor(out=tsh[:], in0=mean[:], in1=sfac[:], op=mybir.AluOpType.mult)
                nc.vector.tensor_tensor(out=tsh[:], in0=beta_sb[:], in1=tsh[:], op=mybir.AluOpType.subtract)

                for t in range(tpc):
                    blk = slice(t * P, (t + 1) * P)
                    fin = p3.tile([P, P], dtype=F32)
                    nc.scalar.activation(
                        fin[:], out_big[:, blk], mybir.ActivationFunctionType.Identity,
                        bias=tsh[:], scale=sfac[:],
                    )
                    nc.sync.dma_start(out=out_d[t], in_=fin[:])

    nc.compile()
    return nc


def _prep(x, edge_index, n_nodes, tpc, slab_target=25088):
    """Host-side sharding: bucket edges by (src slab, dst tile), pad, wrap."""
    n_tiles = N_CORES * tpc
    n_pad = n_tiles * P
    n_tab = ((n_nodes + P - 1) // P) * P
    n_own = tpc * P

    # slab partition of the source-node table
    S = max(1, -(-n_tab // slab_target))
    slab_n = []
    rem = n_tab
    for s in range(S):
        w = min(slab_target, rem)
        slab_n.append(w)
        rem -= w
    assert sum(slab_n) == n_tab
    slab_starts = np.cumsum([0] + slab_n)

    src = np.ascontiguousarray(edge_index[0]).astype(np.int64)
    dst = np.ascontiguousarray(edge_index[1]).astype(np.int64)

    # degree / rowptr over all real edges
    counts_nodes = np.bincount(dst, minlength=n_pad)
    rowptr = np.zeros(n_pad + 1, np.int64)
    np.cumsum(counts_nodes, out=rowptr[1:])
    rowptr_f = rowptr[: n_tab + 1].astype(np.float32)

    core_of = dst // n_own
    slab_of = np.searchsorted(slab_starts, src, side="right") - 1
    tile_of = (dst % n_own) // P

    # per-core per-cell counts -> max over cores
    cell_id = (core_of * S + slab_of) * tpc + tile_of
    cc = np.bincount(cell_id, minlength=N_CORES * S * tpc).reshape(N_CORES, S, tpc)
    counts_max = cc.max(axis=0)
    k_cell, sched, groups = _plan(counts_max, S, tpc)

    gpc = GATHER // CHUNK
    g_tot = sum(groups)

    # per-core edge data, ordered by (slab, tile)
    in_maps = []
    order = np.lexsort((tile_of, slab_of, core_of))
    src_s = src[order]
    dst_s = dst[order]
    slab_s = slab_of[order]
    core_s = core_of[order]
    tile_s = tile_of[order]
    core_bounds = np.searchsorted(core_s, np.arange(N_CORES + 1))

    x_pad = x
    if n_tab > n_nodes:
        x_pad = np.concatenate([x, np.zeros((n_tab - n_nodes, D), np.float32)], 0)

    for c in range(N_CORES):
        lo_e, hi_e = core_bounds[c], core_bounds[c + 1]
        csrc = src_s[lo_e:hi_e]
        cdst = dst_s[lo_e:hi_e]
        cslab = slab_s[lo_e:hi_e]
        ctile = tile_s[lo_e:hi_e]
        lo = c * n_own

        gidx = np.zeros((max(g_tot, 1), P, GATHER // 16), np.int16)
        gdstl = np.full((max(g_tot, 1), P, gpc), -1.0, np.float32)
        goff = 0
        # boundaries of (slab, tile) runs in this core's sorted edge list
        for s in range(S):
            chunks = sched[s]
            n_chunks = len(chunks)
            idx_stream = np.zeros(max(n_chunks, 1) * CHUNK, np.int16)
            dstl_stream = np.full(max(n_chunks, 1) * CHUNK, -1.0, np.float32)
            sm = cslab == s
            ssrc = csrc[sm] - slab_starts[s]
            sdst = cdst[sm]
            stile = ctile[sm]
            pos = 0
            ei = 0
            for t, first, last in chunks:
                if first:
                    # locate this cell's edges
                    cellm = stile == t
                    cell_src = ssrc[cellm]
                    cell_dl = (sdst[cellm] % P).astype(np.float32)
                    ei = 0
                n_take = max(0, min(CHUNK, len(cell_src) - ei))
                if n_take > 0:
                    idx_stream[pos : pos + n_take] = cell_src[ei : ei + n_take]
                    dstl_stream[pos : pos + n_take] = cell_dl[ei : ei + n_take]
                    ei += n_take
                pos += CHUNK
            # pack into gather groups
            for g in range(groups[s]):
                seg = idx_stream[g * GATHER : (g + 1) * GATHER]
                buf = np.zeros(GATHER, np.int16)
                buf[: len(seg)] = seg
                wrapped = buf.reshape(GATHER // 16, 16).T  # [16, GATHER//16]
                gidx[goff + g] = np.tile(wrapped, (8, 1))
                dseg = dstl_stream[g * GATHER : (g + 1) * GATHER]
                dbuf = np.full(GATHER, -1.0, np.float32)
                dbuf[: len(dseg)] = dseg
                gdstl[goff + g] = dbuf.reshape(gpc, CHUNK).T  # [128, gpc]
            goff += groups[s]

        nph = float(max(0, min(lo + n_own, n_pad) - max(lo, n_nodes)))
        xo = x_pad[lo : lo + n_own] if lo + n_own <= n_tab else np.concatenate(
            [x_pad[lo:], np.zeros((lo + n_own - n_tab, D), np.float32)], 0
        )
        rpo = rowptr[lo : lo + n_own + 1].astype(np.float32)
        in_maps.append(
            {
                "x": x_pad,
                "x_own": np.ascontiguousarray(xo),
                "rowptr": rowptr_f.reshape(-1, 1),
                "rowptr_own": rpo.reshape(-1, 1),
                "gidx": gidx,
                "gdstl": gdstl,
                "nphant": np.full((P, 1), nph, np.float32),
            }
        )
    return in_maps, slab_n, sched, groups


def _run(x, edge_index, W, b, gamma, beta, n_nodes, tpc, trace=False, tmpdir=None,
         slab_target=25088, sim=False):
    global LAST_RESULTS
    x = np.ascontiguousarray(x, np.float32)
    in_maps, slab_n, sched, groups = _prep(x, edge_index, n_nodes, tpc, slab_target)
    for m in in_maps:
        m["W"] = np.ascontiguousarray(W, np.float32)
        m["b"] = np.ascontiguousarray(b, np.float32).reshape(D, 1)
        m["gamma"] = np.ascontiguousarray(gamma, np.float32).reshape(D, 1)
        m["beta"] = np.ascontiguousarray(beta, np.float32).reshape(D, 1)
    nc = _build_program(n_nodes, tpc, slab_n, sched, groups)
    if sim:
        from concourse import bass_interp

        msim = bass_interp.MultiCoreSim(nc, N_CORES)
        for c in range(N_CORES):
            for k, v in in_maps[c].items():
                msim.cores[c].tensor(k)[:] = v
        msim.simulate()
        results = [{"out": np.asarray(msim.cores[c].tensor("out"))} for c in range(N_CORES)]
        LAST_RESULTS = None
    else:
        res = run_bass_kernel_spmd(
            nc, in_maps, list(range(N_CORES)), trace=trace, tmpdir=tmpdir
        )
        LAST_RESULTS = res
        results = res.results
    blocks = [
        results[c]["out"].transpose(0, 2, 1).reshape(tpc * P, D) for c in range(N_CORES)
    ]
    return np.concatenate(blocks, axis=0)[:n_nodes]


def kernel(x, edge_index, W, b, gamma, beta):
    x = np.ascontiguousarray(x, np.float32)
    n_nodes = x.shape[0]
    tpc = (n_nodes + N_CORES * P - 1) // (N_CORES * P)
    out = _run(x, edge_index, W, b, gamma, beta, n_nodes, tpc)
    return out.astype(np.float32)


# revision 11
# speedup vs baseline: 1.3402x; 1.0232x over previous
"""GCN conv block (gather -> normalized scatter-add -> matmul -> bias ->
LeakyReLU -> BatchNorm) on 8 Trainium2 NeuronCores.

v2 architecture (per core; SPMD single program, nodes sharded by range):
  - Prologue: dinv = 1/sqrt(1+in_degree) for all nodes from rowptr (device);
    per-core own-range dinv likewise.
  - Main loop over S source slabs. Slab fill: stream x node-tiles, scale by
    dinv (gpsimd), DMA-XBAR-transpose (2x 64-partition halves) into a resident
    f-major slab [128 f, SLAB nodes] in SBUF. Edge gathers then run as big
    gpsimd ap_gather ops (2048 edges each) producing f-major fragments; each
    128-edge chunk is PE-transposed to edge-major and multiplied with an
    on-the-fly one-hot R [e,d] = (dst_local==d) to segment-sum into per-tile
    PSUM, accumulated across slabs in an SBUF A_big [128 f, 12544 d].
  - Phase 3: per destination tile, add the self-loop term (dinv^2 x built via
    the same scale+transpose path from per-core x_own), scale columns by
    dinv[dst] (broadcast tile), apply W, bias, LeakyReLU (relu(z)-0.01relu(-z)
    with fused BN sum/sumsq accumulators).
  - Phase 4: AllReduce BN stats across the 8 cores, apply the affine, store
    out^T per tile. Host reassembles.
Host only shards/pads index structures (CSR bucketing) and reassembles.
"""
import sys

for _p in ("/opt/trn_rl_repo",):
    if _p not in sys.path:
        sys.path.insert(0, _p)

import numpy as np

from concourse import bass, bacc, mybir
import concourse.tile as tile
from concourse.bass_utils import run_bass_kernel_spmd
from concourse.masks import make_identity

P = 128
D = 128
N_CORES = 8
LEAKY = 0.01
BN_EPS = 1e-5
CHUNK = 128          # edges per matmul chunk
GATHER = 2048        # edges per ap_gather instruction
F32 = mybir.dt.float32
I16 = mybir.dt.int16

LAST_RESULTS = None


def _plan(counts_max, S, tpc):
    """counts_max: [S, tpc] max-over-cores edge counts per (slab, tile) cell.
    Returns per-slab chunk schedules: list (per s) of list of (tile, first, last),
    and group counts per slab."""
    k_cell = -(-counts_max // CHUNK)  # ceil; zeros stay zero
    sched = []
    groups = []
    for s in range(S):
        chunks = []
        for t in range(tpc):
            k = int(k_cell[s, t])
            for j in range(k):
                chunks.append((t, j == 0, j == k - 1))
        sched.append(chunks)
        groups.append(-(-len(chunks) // (GATHER // CHUNK)))
    return k_cell, sched, groups


def _build_program(n_nodes, tpc, slab_n, sched, groups):
    n_tab = ((n_nodes + P - 1) // P) * P
    ncol = n_tab // P
    n_own = tpc * P
    S = len(sched)
    g_tot = sum(groups)
    gpc = GATHER // CHUNK  # chunks per gather group

    nc = bacc.Bacc("TRN2", target_bir_lowering=False, debug=False)

    x_d = nc.dram_tensor("x", [n_tab, D], F32, kind="ExternalInput")
    xown_d = nc.dram_tensor("x_own", [n_own, D], F32, kind="ExternalInput")
    rowptr_d = nc.dram_tensor("rowptr", [n_tab + 1, 1], F32, kind="ExternalInput")
    rowptr_own_d = nc.dram_tensor("rowptr_own", [n_own + 1, 1], F32, kind="ExternalInput")
    gidx_d = nc.dram_tensor("gidx", [max(g_tot, 1), P, GATHER // 16], I16, kind="ExternalInput")
    gdstl_d = nc.dram_tensor("gdstl", [max(g_tot, 1), P, gpc], F32, kind="ExternalInput")
    nphant_d = nc.dram_tensor("nphant", [P, 1], F32, kind="ExternalInput")
    w_d = nc.dram_tensor("W", [D, D], F32, kind="ExternalInput")
    b_d = nc.dram_tensor("b", [D, 1], F32, kind="ExternalInput")
    gamma_d = nc.dram_tensor("gamma", [D, 1], F32, kind="ExternalInput")
    beta_d = nc.dram_tensor("beta", [D, 1], F32, kind="ExternalInput")

    out_d = nc.dram_tensor("out", [tpc, D, P], F32, kind="ExternalOutput")

    dinv_full_d = nc.dram_tensor("dinv_full", [n_tab, 1], F32)
    dinv_own_d = nc.dram_tensor("dinv_own", [1, n_own], F32)
    xto_d = nc.dram_tensor("xto", [tpc, D, P], F32)
    cc_in_d = nc.dram_tensor("cc_in", [P, 2], F32)
    cc_out_d = nc.dram_tensor("cc_out", [P, 2], F32)

    inv_n = 1.0 / float(n_nodes)

    with tile.TileContext(nc) as tc:
        with (
            tc.tile_pool(name="persist", bufs=1) as pp,
            tc.tile_pool(name="setup", bufs=1) as sp,
        ):
            # ---------- persistent tiles ----------
            a_big = pp.tile([P, n_own], dtype=F32)
            nc.vector.memset(a_big[:], 0)
            iota_i = sp.tile([P, P], dtype=mybir.dt.int32)
            nc.gpsimd.iota(iota_i[:], pattern=[[1, P]], base=0, channel_multiplier=0)
            iota_f = pp.tile([P, P], dtype=F32)
            nc.vector.tensor_copy(iota_f[:], iota_i[:])
            ident = pp.tile([P, P], dtype=F32)
            make_identity(nc, ident[:])
            w_sb = pp.tile([D, D], dtype=F32)
            nc.sync.dma_start(out=w_sb[:], in_=w_d[:])
            b_sb = pp.tile([D, 1], dtype=F32)
            nc.sync.dma_start(out=b_sb[:], in_=b_d[:])
            nb_sb = pp.tile([D, 1], dtype=F32)
            nc.vector.tensor_scalar(nb_sb[:], b_sb[:], -1.0, None, mybir.AluOpType.mult)
            gamma_sb = pp.tile([D, 1], dtype=F32)
            nc.sync.dma_start(out=gamma_sb[:], in_=gamma_d[:])
            beta_sb = pp.tile([D, 1], dtype=F32)
            nc.sync.dma_start(out=beta_sb[:], in_=beta_d[:])
            nph_sb = pp.tile([P, 1], dtype=F32)
            nc.sync.dma_start(out=nph_sb[:], in_=nphant_d[:])
            spos = pp.tile([P, tpc], dtype=F32)
            sneg = pp.tile([P, tpc], dtype=F32)
            qpos = pp.tile([P, tpc], dtype=F32)
            qneg = pp.tile([P, tpc], dtype=F32)

            # ---------- prologue A: dinv tables ----------
            rp0 = sp.tile([P, ncol], dtype=F32)
            rp1 = sp.tile([P, ncol], dtype=F32)
            rpf = rowptr_d[:].flatten()
            nc.sync.dma_start(out=rp0[:], in_=rpf[0:n_tab].rearrange("(p c) -> p c", p=P))
            nc.sync.dma_start(out=rp1[:], in_=rpf[1 : n_tab + 1].rearrange("(p c) -> p c", p=P))
            degm = sp.tile([P, ncol], dtype=F32)
            nc.vector.tensor_tensor(out=degm[:], in0=rp1[:], in1=rp0[:], op=mybir.AluOpType.subtract)
            sq = sp.tile([P, ncol], dtype=F32)
            nc.scalar.activation(sq[:], degm[:], mybir.ActivationFunctionType.Sqrt, bias=1.0)
            dinv_sb = sp.tile([P, ncol], dtype=F32)
            nc.vector.reciprocal(dinv_sb[:], sq[:])
            nc.sync.dma_start(
                out=dinv_full_d[:].flatten()[0:n_tab].rearrange("(p c) -> p c", p=P),
                in_=dinv_sb[:],
            )
            ro0 = sp.tile([P, tpc], dtype=F32)
            ro1 = sp.tile([P, tpc], dtype=F32)
            rof = rowptr_own_d[:].flatten()
            nc.sync.dma_start(out=ro0[:], in_=rof[0:n_own].rearrange("(p c) -> p c", p=P))
            nc.sync.dma_start(out=ro1[:], in_=rof[1 : n_own + 1].rearrange("(p c) -> p c", p=P))
            degmo = sp.tile([P, tpc], dtype=F32)
            nc.vector.tensor_tensor(out=degmo[:], in0=ro1[:], in1=ro0[:], op=mybir.AluOpType.subtract)
            sqo = sp.tile([P, tpc], dtype=F32)
            nc.scalar.activation(sqo[:], degmo[:], mybir.ActivationFunctionType.Sqrt, bias=1.0)
            dinvo_sb = pp.tile([P, tpc], dtype=F32)
            nc.vector.reciprocal(dinvo_sb[:], sqo[:])
            nc.sync.dma_start(
                out=dinv_own_d[:].flatten().rearrange("(p c) -> p c", p=P),
                in_=dinvo_sb[:],
            )

            # ---------- prologue C: x_own scaled+transposed tiles (self term) ----------
            with (
                tc.tile_pool(name="pc_in", bufs=3) as pcin,
                tc.tile_pool(name="pc_tr", bufs=3) as pctr,
                tc.tile_pool(name="pc_ps", bufs=3, space="PSUM") as pcps,
            ):
                # per-tile dinv_own as [128,1]: dinvo_sb is laid out (p, c) = node p*tpc+c,
                # which does NOT match tile-slicing; reload per tile from dinv_own_d.
                for t in range(tpc):
                    xo = pcin.tile([P, D], dtype=F32)
                    nc.sync.dma_start(out=xo[:], in_=xown_d[t * P : (t + 1) * P, :])
                    dv = pcin.tile([P, 1], dtype=F32)
                    nc.sync.dma_start(
                        out=dv[:], in_=dinv_own_d[:].flatten()[t * P : (t + 1) * P][:, None]
                    )
                    xs = pcin.tile([P, D], dtype=F32)
                    nc.vector.tensor_scalar(xs[:], xo[:], dv[:], None, mybir.AluOpType.mult)
                    xtp = pcps.tile([P, P], dtype=F32, space="PSUM")
                    nc.tensor.transpose(out=xtp[:], in_=xs[:], identity=ident[:])
                    xt = pctr.tile([P, P], dtype=F32)
                    nc.scalar.activation(xt[:], xtp[:], mybir.ActivationFunctionType.Copy)
                    nc.sync.dma_start(out=xto_d[t], in_=xt[:])

            # ---------- main: slabs ----------
            goff = 0
            with (
                tc.tile_pool(name="slab", bufs=1) as slabp,
                tc.tile_pool(name="fill", bufs=3) as fillp,
                tc.tile_pool(name="frag", bufs=2) as fragp,
                tc.tile_pool(name="gmeta", bufs=2) as gmeta,
                tc.tile_pool(name="rps", bufs=3) as rpool,
                tc.tile_pool(name="trs", bufs=3) as trpool,
                tc.tile_pool(name="pst", bufs=2, space="PSUM") as pst,
                tc.tile_pool(name="psc", bufs=2, space="PSUM") as psc,
                tc.tile_pool(name="fps", bufs=2, space="PSUM") as fps,
            ):
                BF = mybir.dt.bfloat16
                slab_sb = slabp.tile([P, slab_n[0]], dtype=F32)
                for s in range(S):
                    base = sum(slab_n[:s])
                    width = slab_n[s]
                    ntile = width // P
                    # fill slab: contiguous block loads + scale + PE-transpose
                    FB = 4
                    nblk = -(-ntile // FB)
                    for jb in range(nblk):
                        j0 = jb * FB
                        nt = min(FB, ntile - j0)
                        g = base + j0 * P
                        xin = fillp.tile([P, nt, D], dtype=F32)
                        nc.sync.dma_start(
                            out=xin[:],
                            in_=x_d[g : g + nt * P, :].rearrange("(p c) f -> p c f", p=P),
                        )
                        dv = fillp.tile([P, nt], dtype=F32)
                        nc.scalar.dma_start(
                            out=dv[:],
                            in_=dinv_full_d[:].flatten()[g : g + nt * P].rearrange(
                                "(p c) -> p c", p=P
                            ),
                        )
                        xs = fillp.tile([P, nt, D], dtype=F32)
                        nc.vector.tensor_tensor(
                            out=xs[:], in0=xin[:],
                            in1=dv[:, :, None].to_broadcast([P, nt, D]),
                            op=mybir.AluOpType.mult,
                        )
                        fpp = fps.tile([P, FB * P], dtype=F32, space="PSUM")
                        for k in range(nt):
                            nc.tensor.transpose(
                                out=fpp[:, k * P : (k + 1) * P], in_=xs[:, k, :],
                                identity=ident[:],
                            )
                        nc.scalar.activation(
                            slab_sb[:, j0 * P : (j0 + nt) * P], fpp[:, : nt * P],
                            mybir.ActivationFunctionType.Copy,
                        )
                    # gather + chunk pipeline (quads of 4 chunks)
                    chunks = sched[s]
                    cell_ps = None
                    for g in range(groups[s]):
                        gi = gmeta.tile([P, GATHER // 16], dtype=I16)
                        nc.scalar.dma_start(out=gi[:], in_=gidx_d[goff + g])
                        gd = gmeta.tile([P, gpc], dtype=F32)
                        nc.scalar.dma_start(out=gd[:], in_=gdstl_d[goff + g])
                        frag = fragp.tile([P, GATHER], dtype=F32)
                        nc.gpsimd.ap_gather(
                            out_ap=frag[:],
                            in_ap=slab_sb[:, 0:width],
                            idxs_ap=gi[:],
                            channels=P,
                            num_elems=width,
                            d=1,
                            num_idxs=GATHER,
                        )
                        ng = min(gpc, len(chunks) - g * gpc)
                        for q in range(-(-ng // 4)):
                            j0 = q * 4
                            nj = min(4, ng - j0)
                            trp = pst.tile([P, 4 * P], dtype=F32, space="PSUM")
                            for jj in range(nj):
                                j = j0 + jj
                                nc.tensor.transpose(
                                    out=trp[:, jj * P : (jj + 1) * P],
                                    in_=frag[:, j * P : (j + 1) * P],
                                    identity=ident[:],
                                )
                            trq = trpool.tile([P, 4 * P], dtype=BF)
                            nc.scalar.activation(
                                trq[:, : nj * P], trp[:, : nj * P],
                                mybir.ActivationFunctionType.Copy,
                            )
                            trl = trpool.tile([P, 4 * P], dtype=BF)
                            nc.vector.tensor_tensor(
                                out=trl[:, : nj * P], in0=trp[:, : nj * P],
                                in1=trq[:, : nj * P], op=mybir.AluOpType.subtract,
                            )
                            rq = rpool.tile([P, 4, P], dtype=BF)
                            nc.vector.tensor_tensor(
                                out=rq[:, :nj, :],
                                in0=iota_f[:, None, :].to_broadcast([P, nj, P]),
                                in1=gd[:, j0 : j0 + nj][:, :, None].to_broadcast([P, nj, P]),
                                op=mybir.AluOpType.is_equal,
                            )
                            for jj in range(nj):
                                ci = g * gpc + j0 + jj
                                t, first, last = chunks[ci]
                                if first:
                                    cell_ps = psc.tile([P, P], dtype=F32, space="PSUM")
                                nc.tensor.matmul(
                                    out=cell_ps[:],
                                    lhsT=trq[:, jj * P : (jj + 1) * P],
                                    rhs=rq[:, jj, :],
                                    start=first, stop=False,
                                )
                                nc.tensor.matmul(
                                    out=cell_ps[:],
                                    lhsT=trl[:, jj * P : (jj + 1) * P],
                                    rhs=rq[:, jj, :],
                                    start=False, stop=last,
                                )
                                if last:
                                    blk = slice(t * P, (t + 1) * P)
                                    nc.vector.tensor_tensor(
                                        out=a_big[:, blk], in0=a_big[:, blk], in1=cell_ps[:],
                                        op=mybir.AluOpType.add,
                                    )
                    goff += groups[s]

            # ---------- phase 3: per-tile self + dinv_dst + W + bias + leaky ----------
            with (
                tc.tile_pool(name="dinvb", bufs=1) as dbp,
                tc.tile_pool(name="outb", bufs=1) as obp,
                tc.tile_pool(name="p3", bufs=3) as p3,
                tc.tile_pool(name="ps3", bufs=2, space="PSUM") as ps3,
            ):
                dinvb = dbp.tile([P, n_own], dtype=F32)
                nc.sync.dma_start(out=dinvb[:], in_=dinv_own_d[0:1, :].to_broadcast([P, n_own]))
                out_big = obp.tile([P, n_own], dtype=F32)
                for t in range(tpc):
                    blk = slice(t * P, (t + 1) * P)
                    xo = p3.tile([P, P], dtype=F32)
                    nc.sync.dma_start(out=xo[:], in_=xto_d[t])
                    at = p3.tile([P, P], dtype=F32)
                    nc.vector.tensor_tensor(
                        out=at[:], in0=a_big[:, blk], in1=xo[:], op=mybir.AluOpType.add
                    )
                    nc.vector.tensor_tensor(
                        out=at[:], in0=at[:], in1=dinvb[:, blk], op=mybir.AluOpType.mult
                    )
                    pc3 = ps3.tile([P, P], dtype=F32, space="PSUM")
                    nc.tensor.matmul(out=pc3[:], lhsT=w_sb[:], rhs=at[:], start=True, stop=True)
                    pos = p3.tile([P, P], dtype=F32)
                    neg = p3.tile([P, P], dtype=F32)
                    scr = p3.tile([P, P], dtype=F32)
                    scr2 = p3.tile([P, P], dtype=F32)
                    nc.scalar.activation(
                        pos[:], pc3[:], mybir.ActivationFunctionType.Relu,
                        bias=b_sb[:], scale=1.0, accum_out=spos[:, t : t + 1],
                    )
                    nc.scalar.activation(
                        neg[:], pc3[:], mybir.ActivationFunctionType.Relu,
                        bias=nb_sb[:], scale=-1.0, accum_out=sneg[:, t : t + 1],
                    )
                    nc.scalar.activation(
                        scr[:], pos[:], mybir.ActivationFunctionType.Square,
                        accum_out=qpos[:, t : t + 1],
                    )
                    nc.scalar.activation(
                        scr2[:], neg[:], mybir.ActivationFunctionType.Square,
                        accum_out=qneg[:, t : t + 1],
                    )
                    nc.gpsimd.tensor_scalar(neg[:], neg[:], LEAKY, None, mybir.AluOpType.mult)
                    nc.gpsimd.tensor_tensor(
                        out=out_big[:, blk], in0=pos[:], in1=neg[:], op=mybir.AluOpType.subtract
                    )

                # ---------- phase 4: BN stats + AllReduce + affine + store ----------
                rsp = sp.tile([P, 1], dtype=F32)
                rsn = sp.tile([P, 1], dtype=F32)
                rqp = sp.tile([P, 1], dtype=F32)
                rqn = sp.tile([P, 1], dtype=F32)
                nc.vector.tensor_reduce(rsp[:], spos[:], mybir.AxisListType.X, mybir.AluOpType.add)
                nc.vector.tensor_reduce(rsn[:], sneg[:], mybir.AxisListType.X, mybir.AluOpType.add)
                nc.vector.tensor_reduce(rqp[:], qpos[:], mybir.AxisListType.X, mybir.AluOpType.add)
                nc.vector.tensor_reduce(rqn[:], qneg[:], mybir.AxisListType.X, mybir.AluOpType.add)
                s_lr = sp.tile([P, 1], dtype=F32)
                nc.vector.tensor_scalar(s_lr[:], rsn[:], -LEAKY, None, mybir.AluOpType.mult)
                nc.vector.tensor_tensor(out=s_lr[:], in0=s_lr[:], in1=rsp[:], op=mybir.AluOpType.add)
                q_lr = sp.tile([P, 1], dtype=F32)
                nc.vector.tensor_scalar(q_lr[:], rqn[:], LEAKY * LEAKY, None, mybir.AluOpType.mult)
                nc.vector.tensor_tensor(out=q_lr[:], in0=q_lr[:], in1=rqp[:], op=mybir.AluOpType.add)
                pb = sp.tile([P, 1], dtype=F32)
                nb2 = sp.tile([P, 1], dtype=F32)
                nc.scalar.activation(pb[:], b_sb[:], mybir.ActivationFunctionType.Relu)
                nc.scalar.activation(nb2[:], b_sb[:], mybir.ActivationFunctionType.Relu, scale=-1.0)
                lb = sp.tile([P, 1], dtype=F32)
                nc.vector.tensor_scalar(lb[:], nb2[:], -LEAKY, None, mybir.AluOpType.mult)
                nc.vector.tensor_tensor(out=lb[:], in0=lb[:], in1=pb[:], op=mybir.AluOpType.add)
                lb2 = sp.tile([P, 1], dtype=F32)
                nc.scalar.activation(lb2[:], lb[:], mybir.ActivationFunctionType.Square)
                corr = sp.tile([P, 1], dtype=F32)
                nc.vector.tensor_tensor(out=corr[:], in0=nph_sb[:], in1=lb[:], op=mybir.AluOpType.mult)
                nc.vector.tensor_tensor(out=s_lr[:], in0=s_lr[:], in1=corr[:], op=mybir.AluOpType.subtract)
                nc.vector.tensor_tensor(out=corr[:], in0=nph_sb[:], in1=lb2[:], op=mybir.AluOpType.mult)
                nc.vector.tensor_tensor(out=q_lr[:], in0=q_lr[:], in1=corr[:], op=mybir.AluOpType.subtract)

                cc_sb = sp.tile([P, 2], dtype=F32)
                nc.vector.tensor_copy(cc_sb[:, 0:1], s_lr[:])
                nc.vector.tensor_copy(cc_sb[:, 1:2], q_lr[:])
                nc.sync.dma_start(out=cc_in_d[:], in_=cc_sb[:])
                nc.gpsimd.collective_compute(
                    "AllReduce",
                    mybir.AluOpType.add,
                    replica_groups=[list(range(N_CORES))],
                    ins=[cc_in_d[:]],
                    outs=[cc_out_d[:]],
                )
                st = sp.tile([P, 2], dtype=F32)
                nc.sync.dma_start(out=st[:], in_=cc_out_d[:])
                mean = sp.tile([P, 1], dtype=F32)
                nc.vector.tensor_scalar(mean[:], st[:, 0:1], inv_n, None, mybir.AluOpType.mult)
                msq = sp.tile([P, 1], dtype=F32)
                nc.vector.tensor_scalar(msq[:], st[:, 1:2], inv_n, None, mybir.AluOpType.mult)
                m2 = sp.tile([P, 1], dtype=F32)
                nc.scalar.activation(m2[:], mean[:], mybir.ActivationFunctionType.Square)
                var = sp.tile([P, 1], dtype=F32)
                nc.vector.tensor_tensor(out=var[:], in0=msq[:], in1=m2[:], op=mybir.AluOpType.subtract)
                nc.vector.tensor_scalar(var[:], var[:], BN_EPS, None, mybir.AluOpType.add)
                sd = sp.tile([P, 1], dtype=F32)
                nc.scalar.activation(sd[:], var[:], mybir.ActivationFunctionType.Sqrt)
                rstd = sp.tile([P, 1], dtype=F32)
                nc.vector.reciprocal(rstd[:], sd[:])
                sfac = sp.tile([P, 1], dtype=F32)
                nc.vector.tensor_tensor(out=sfac[:], in0=gamma_sb[:], in1=rstd[:], op=mybir.AluOpType.mult)
                tsh = sp.tile([P, 1], dtype=F32)
                nc.vector.tensor_tensor(out=tsh[:], in0=mean[:], in1=sfac[:], op=mybir.AluOpType.mult)
                nc.vector.tensor_tensor(out=tsh[:], in0=beta_sb[:], in1=tsh[:], op=mybir.AluOpType.subtract)

                for t in range(tpc):
                    blk = slice(t * P, (t + 1) * P)
                    fin = p3.tile([P, P], dtype=F32)
                    nc.scalar.activation(
                        fin[:], out_big[:, blk], mybir.ActivationFunctionType.Identity,
                        bias=tsh[:], scale=sfac[:],
                    )
                    nc.sync.dma_start(out=out_d[t], in_=fin[:])

    nc.compile()
    return nc


def _prep(x, edge_index, n_nodes, tpc, slab_target=25088):
    """Host-side sharding: bucket edges by (src slab, dst tile), pad, wrap."""
    n_tiles = N_CORES * tpc
    n_pad = n_tiles * P
    n_tab = ((n_nodes + P - 1) // P) * P
    n_own = tpc * P

    # slab partition of the source-node table
    S = max(1, -(-n_tab // slab_target))
    slab_n = []
    rem = n_tab
    for s in range(S):
        w = min(slab_target, rem)
        slab_n.append(w)
        rem -= w
    assert sum(slab_n) == n_tab
    slab_starts = np.cumsum([0] + slab_n)

    src = np.ascontiguousarray(edge_index[0]).astype(np.int64)
    dst = np.ascontiguousarray(edge_index[1]).astype(np.int64)

    # degree / rowptr over all real edges
    counts_nodes = np.bincount(dst, minlength=n_pad)
    rowptr = np.zeros(n_pad + 1, np.int64)
    np.cumsum(counts_nodes, out=rowptr[1:])
    rowptr_f = rowptr[: n_tab + 1].astype(np.float32)

    core_of = dst // n_own
    slab_of = np.searchsorted(slab_starts, src, side="right") - 1
    tile_of = (dst % n_own) // P

    # fill stores node (4p+c) of each 512-row block at slab column c*128+p;
    # precompute per-slab permutation: local node offset -> slab column
    perm_col = np.empty(n_tab, np.int64)
    for s in range(S):
        w = slab_n[s]
        ntile = w // P
        loc = np.arange(w)
        blk = loc // (4 * P)
        r = loc % (4 * P)
        nt = np.minimum(4, ntile - blk * 4)
        perm_col[slab_starts[s] : slab_starts[s] + w] = (
            blk * 4 * P + (r % nt) * P + r // nt
        )

    # per-core per-cell counts -> max over cores
    cell_id = (core_of * S + slab_of) * tpc + tile_of
    cc = np.bincount(cell_id, minlength=N_CORES * S * tpc).reshape(N_CORES, S, tpc)
    counts_max = cc.max(axis=0)
    k_cell, sched, groups = _plan(counts_max, S, tpc)

    gpc = GATHER // CHUNK
    g_tot = sum(groups)

    # per-core edge data, ordered by (slab, tile)
    in_maps = []
    order = np.lexsort((tile_of, slab_of, core_of))
    src_s = src[order]
    dst_s = dst[order]
    slab_s = slab_of[order]
    core_s = core_of[order]
    tile_s = tile_of[order]
    core_bounds = np.searchsorted(core_s, np.arange(N_CORES + 1))

    x_pad = x
    if n_tab > n_nodes:
        x_pad = np.concatenate([x, np.zeros((n_tab - n_nodes, D), np.float32)], 0)

    for c in range(N_CORES):
        lo_e, hi_e = core_bounds[c], core_bounds[c + 1]
        csrc = src_s[lo_e:hi_e]
        cdst = dst_s[lo_e:hi_e]
        cslab = slab_s[lo_e:hi_e]
        ctile = tile_s[lo_e:hi_e]
        lo = c * n_own

        gidx = np.zeros((max(g_tot, 1), P, GATHER // 16), np.int16)
        gdstl = np.full((max(g_tot, 1), P, gpc), -1.0, np.float32)
        goff = 0
        # boundaries of (slab, tile) runs in this core's sorted edge list
        for s in range(S):
            chunks = sched[s]
            n_chunks = len(chunks)
            idx_stream = np.zeros(max(n_chunks, 1) * CHUNK, np.int16)
            dstl_stream = np.full(max(n_chunks, 1) * CHUNK, -1.0, np.float32)
            sm = cslab == s
            ssrc = perm_col[csrc[sm]]
            sdst = cdst[sm]
            stile = ctile[sm]
            pos = 0
            ei = 0
            for t, first, last in chunks:
                if first:
                    # locate this cell's edges
                    cellm = stile == t
                    cell_src = ssrc[cellm]
                    cell_dl = (sdst[cellm] % P).astype(np.float32)
                    ei = 0
                n_take = max(0, min(CHUNK, len(cell_src) - ei))
                if n_take > 0:
                    idx_stream[pos : pos + n_take] = cell_src[ei : ei + n_take]
                    dstl_stream[pos : pos + n_take] = cell_dl[ei : ei + n_take]
                    ei += n_take
                pos += CHUNK
            # pack into gather groups
            for g in range(groups[s]):
                seg = idx_stream[g * GATHER : (g + 1) * GATHER]
                buf = np.zeros(GATHER, np.int16)
                buf[: len(seg)] = seg
                wrapped = buf.reshape(GATHER // 16, 16).T  # [16, GATHER//16]
                gidx[goff + g] = np.tile(wrapped, (8, 1))
                dseg = dstl_stream[g * GATHER : (g + 1) * GATHER]
                dbuf = np.full(GATHER, -1.0, np.float32)
                dbuf[: len(dseg)] = dseg
                gdstl[goff + g] = dbuf.reshape(gpc, CHUNK).T  # [128, gpc]
            goff += groups[s]

        nph = float(max(0, min(lo + n_own, n_pad) - max(lo, n_nodes)))
        xo = x_pad[lo : lo + n_own] if lo + n_own <= n_tab else np.concatenate(
            [x_pad[lo:], np.zeros((lo + n_own - n_tab, D), np.float32)], 0
        )
        rpo = rowptr[lo : lo + n_own + 1].astype(np.float32)
        in_maps.append(
            {
                "x": x_pad,
                "x_own": np.ascontiguousarray(xo),
                "rowptr": rowptr_f.reshape(-1, 1),
                "rowptr_own": rpo.reshape(-1, 1),
                "gidx": gidx,
                "gdstl": gdstl,
                "nphant": np.full((P, 1), nph, np.float32),
            }
        )
    return in_maps, slab_n, sched, groups


def _run(x, edge_index, W, b, gamma, beta, n_nodes, tpc, trace=False, tmpdir=None,
         slab_target=25088, sim=False):
    global LAST_RESULTS
    x = np.ascontiguousarray(x, np.float32)
    in_maps, slab_n, sched, groups = _prep(x, edge_index, n_nodes, tpc, slab_target)
    for m in in_maps:
        m["W"] = np.ascontiguousarray(W, np.float32)
        m["b"] = np.ascontiguousarray(b, np.float32).reshape(D, 1)
        m["gamma"] = np.ascontiguousarray(gamma, np.float32).reshape(D, 1)
        m["beta"] = np.ascontiguousarray(beta, np.float32).reshape(D, 1)
    nc = _build_program(n_nodes, tpc, slab_n, sched, groups)
    if sim:
        from concourse import bass_interp

        msim = bass_interp.MultiCoreSim(nc, N_CORES)
        for c in range(N_CORES):
            for k, v in in_maps[c].items():
                msim.cores[c].tensor(k)[:] = v
        msim.simulate()
        results = [{"out": np.asarray(msim.cores[c].tensor("out"))} for c in range(N_CORES)]
        LAST_RESULTS = None
    else:
        res = run_bass_kernel_spmd(
            nc, in_maps, list(range(N_CORES)), trace=trace, tmpdir=tmpdir
        )
        LAST_RESULTS = res
        results = res.results
    blocks = [
        results[c]["out"].transpose(0, 2, 1).reshape(tpc * P, D) for c in range(N_CORES)
    ]
    return np.concatenate(blocks, axis=0)[:n_nodes]


def kernel(x, edge_index, W, b, gamma, beta):
    x = np.ascontiguousarray(x, np.float32)
    n_nodes = x.shape[0]
    tpc = (n_nodes + N_CORES * P - 1) // (N_CORES * P)
    out = _run(x, edge_index, W, b, gamma, beta, n_nodes, tpc)
    return out.astype(np.float32)
